# revision 18
# baseline (speedup 1.0000x reference)
"""Local (windowed) attention with shared KV head — TRN2 Bass kernel.

Problem: b=1, L=4096, d_model=1024, n_head=16, d_head=64, w=512.
  qp = (q@Wq)/8; k,v = kv@Wkv; per 512-chunk attention over {prev,self,next}
  chunks with zero-padded edges (softmax includes exp(0)=1 terms for pads);
  out = ctx @ Wo.

Sharding: sequence-parallel over the 8 chunks, one chunk per NeuronCore.
Each core recomputes the K/V projection for its 3-chunk halo (no
collectives). Edge cores receive zero-filled halo slices, which reproduces
the reference's zero-padding exactly (scores 0 -> exp 1 in the softmax).

v2 (this file): all-bf16 datapath (f32 PSUM accumulation), fast softmax
normalization (reciprocal_approx_fast on the PSUM denominator row + DVE
partition-broadcast multiply), PE warm-up matmuls under the initial DMA
wait, q-projection interleaved into the attention loop, v-transposes via
the DMA XBAR instead of the PE.

Per-core dataflow:
  kvp^T = [Wv|Wk]^T @ kv^T            (24 MMs)   -> vT (rows 0:64), kT (64:128)
  k3T2  = kT duplicated to both partition halves (SBUF->SBUF DMA)
  v65   = DMA-transpose(vT) with a ones column appended   ([y,64+1] tiles)
  qp^T  = (Wq/8)^T @ q^T              (64 MMs)   -> 8 tiles [128,512], head pair per tile
  scores: S^T[y,x] per head, row-packed pairs (2 heads share the PE array)
  P^T   = exp(S^T) on ScalarE, PSUM->SBUF bf16, [128,1024] groups
  ctx^T+Z = [v|1]^T @ P^T fused       (M=65: rows 0:64 ctx, row 64 = softmax denom)
  norm  : zinv=recip_approx(Z row); ctxn = ctx * zinv (partition-broadcast)
  out   = ctxn^T-tiles (lhsT) @ Wo    (64 MMs)   -> [512,1024] row-major -> DMA
"""

import numpy as np

B, L, DM, NH, DH, W = 1, 4096, 1024, 16, 64, 512
NCORES = 8
CH = L // NCORES        # 512 tokens per core
YW = 3 * W              # 1536 halo positions
P = 128
NF = DM // P            # 8 feature tiles
NY = YW // P            # 12 y tiles
NPAIR = NH // 2         # 8 head pairs
NGRP = NY // 2          # 6 score groups of 2 y-tiles

_CACHE = {}


def _build():
    import concourse.mybir as mybir
    import concourse.tile as tile
    from concourse import bacc
    from concourse.masks import make_identity
    from contextlib import ExitStack

    F32 = mybir.dt.float32
    F32R = mybir.dt.float32r
    BF16 = mybir.dt.bfloat16
    EXP = mybir.ActivationFunctionType.Exp

    nc = bacc.Bacc("TRN2", target_bir_lowering=False, debug=False)
    QT = nc.dram_tensor("QT", [DM, CH], BF16, kind="ExternalInput")
    KVT = nc.dram_tensor("KVT", [DM, YW], BF16, kind="ExternalInput")
    WQ = nc.dram_tensor("WQ", [DM, DM], BF16, kind="ExternalInput")     # pre-scaled by 1/8
    WVK = nc.dram_tensor("WVK", [DM, P], BF16, kind="ExternalInput")    # [Wv | Wk]
    WO = nc.dram_tensor("WO", [DM, DM], BF16, kind="ExternalInput")
    OUT = nc.dram_tensor("OUT", [CH, DM], F32, kind="ExternalOutput")

    with tile.TileContext(nc) as tc, ExitStack() as ctx:
        perm = ctx.enter_context(tc.tile_pool(name="perm", bufs=1))

        identb = perm.tile([P, P], BF16, tag="identb")
        make_identity(nc, identb[:])
        warmsb = perm.tile([P, W], BF16, tag="warmsb")
        nc.vector.memset(warmsb[:], 1.0)
        zw = perm.tile([1, 16], F32, tag="zw")
        nc.vector.memset(zw[:], 0.0)
        onesr = perm.tile([1, 64], F32, tag="onesr")
        nc.vector.memset(onesr[:], 1.0)
        zwo = perm.tile([1, 16], F32, tag="zwo")
        # early exp-table load on ScalarE (runs during the initial DMA wait)
        nc.scalar.activation(zwo[:], zw[:], EXP)

        # --- persistent SBUF tiles
        wvk = [perm.tile([P, P], BF16, tag=f"wvk{f}", name=f"wvk{f}") for f in range(NF)]
        wq = [perm.tile([P, DM], BF16, tag=f"wq{f}", name=f"wq{f}") for f in range(NF)]
        wo = [perm.tile([P, DM], BF16, tag=f"wo{f}", name=f"wo{f}") for f in range(NF)]
        qt = [perm.tile([P, CH], BF16, tag=f"qt{f}", name=f"qt{f}") for f in range(NF)]
        k3T2 = perm.tile([P, YW], BF16, tag="k3T2")
        vTs = perm.tile([64, YW], BF16, tag="vTs")
        v65 = [perm.tile([P, 65], BF16, tag=f"v65_{t}", name=f"v65_{t}") for t in range(NY)]
        qpT = [perm.tile([P, CH], BF16, tag=f"qpT{m}", name=f"qpT{m}") for m in range(NF)]
        ctxn = [perm.tile([P, CH], BF16, tag=f"ctxn{i}", name=f"ctxn{i}") for i in range(NPAIR)]
        zi = [perm.tile([1, W], F32, tag=f"zi{h}", name=f"zi{h}") for h in range(NH)]
        zs = [perm.tile([1, W], F32, tag=f"zs{h}", name=f"zs{h}") for h in range(NH)]
        outacc = [perm.tile([P, W], F32, tag=f"oa{j}", name=f"oa{j}")
                  for j in range(8)]

        for f in range(NF):
            nc.sync.dma_start(wvk[f][:], WVK.ap()[P * f:P * (f + 1), :])

        with tc.tile_pool(name="kvt", bufs=1) as kvtp, \
             tc.tile_pool(name="warm", bufs=1, space="PSUM") as wmp, \
             tc.tile_pool(name="ph0ps", bufs=2, space="PSUM") as ph0:
            kvt = [kvtp.tile([P, YW], BF16, tag=f"kvt{f}", name=f"kvt{f}") for f in range(NF)]
            # issue ALL input DMAs up front so the sync queue never blocks a
            # load behind compute-dependent work; KVT (needed first) leads,
            # then QT and the first q-projection's WQ columns
            for n in range(3):
                for f in range(NF):
                    ns = slice(W * n, W * (n + 1))
                    nc.sync.dma_start(kvt[f][:, ns], KVT.ap()[P * f:P * (f + 1), ns])
            for f in range(NF):
                nc.sync.dma_start(qt[f][:], QT.ap()[P * f:P * (f + 1), :])
            for f in range(NF):
                nc.sync.dma_start(wq[f][:, 0:2 * P], WQ.ap()[P * f:P * (f + 1), 0:2 * P])
            for f in range(NF):
                nc.sync.dma_start(wq[f][:, 2 * P:], WQ.ap()[P * f:P * (f + 1), 2 * P:])
            for f in range(NF):
                nc.sync.dma_start(wo[f][:], WO.ap()[P * f:P * (f + 1), :])
            # PE warm-up: dense accumulating matmuls over dummy data keep the
            # HAM activity monitor busy while the KVT DMA lands (K=8/8 sooner)
            wps = wmp.tile([P, W], F32, tag="wps")
            for k in range(16):
                nc.tensor.matmul(wps[:], identb[:], warmsb[:],
                                 start=(k == 0), stop=(k == 15))
            # kv projection: [128,512] psum per n-tile; rows 0:64=vT, 64:128=kT
            for n in range(3):
                ps = ph0.tile([P, W], F32, tag="kvp")
                for f in range(NF):
                    nc.tensor.matmul(ps[:], wvk[f][:], kvt[f][:, W * n:W * (n + 1)],
                                     start=(f == 0), stop=(f == NF - 1))
                ns = slice(W * n, W * (n + 1))
                with nc.allow_low_precision(reason="bf16 datapath"):
                    nc.vector.tensor_copy(vTs[:, ns], ps[0:64, :])
                    nc.vector.tensor_copy(k3T2[64:128, ns], ps[64:128, :])
                # v65 tiles for this chunk: DMA-XBAR transpose of vT slices
                for t in range(4 * n, 4 * n + 4):
                    nc.sync.dma_start_transpose(v65[t][:, 0:64],
                                                vTs[:, P * t:P * (t + 1)])
                    nc.vector.memset(v65[t][:, 64:65], 1.0)
            # duplicate kT into the low partition half (partition remap DMA)
            nc.sync.dma_start(k3T2[0:64, :], k3T2[64:128, :])
            # second warm-up burst: bridges the PE-idle window between the kv
            # projection and the q projection (QT/WQ DMA still in flight)
            wps2 = wmp.tile([P, W], F32, tag="wps2")
            for k in range(12):
                nc.tensor.matmul(wps2[:], identb[:], warmsb[:],
                                 start=(k == 0), stop=(k == 11))

        def qproj(m, pool):
            ps = pool.tile([P, CH], F32, tag="mis")
            for f in range(NF):
                nc.tensor.matmul(ps[:], wq[f][:, P * m:P * (m + 1)], qt[f][:],
                                 start=(f == 0), stop=(f == NF - 1))
            with nc.allow_low_precision(reason="bf16 datapath"):
                nc.vector.tensor_copy(qpT[m][:], ps[:])

        # --- q projection for the first two pairs
        with tc.tile_pool(name="qpps", bufs=2, space="PSUM") as qpp:
            qproj(0, qpp)
            qproj(1, qpp)

        # --- attention per head pair; remaining q projections interleaved
        with tc.tile_pool(name="scps", bufs=2, space="PSUM") as scp, \
             tc.tile_pool(name="cxps", bufs=3, space="PSUM") as cxp, \
             tc.tile_pool(name="msps", bufs=1, space="PSUM") as msp, \
             tc.tile_pool(name="pt", bufs=4) as ptp:
            def outpart(j):
                # partial output projection over pairs 0..5 (PE filler during
                # the last two pairs' attention; ACT stays the critical path)
                x, o = divmod(j, 2)
                xs = slice(P * x, P * (x + 1))
                os_ = slice(W * o, W * (o + 1))
                ps = msp.tile([P, W], F32, tag="mis")
                for ii in range(6):
                    nc.tensor.matmul(ps[:], ctxn[ii][:, xs], wo[ii][:, os_],
                                     start=(ii == 0), stop=(ii == 5))
                nc.vector.tensor_copy(outacc[j][:], ps[:])

            for i in range(NPAIR):
                cxA = cxp.tile([P, W], F32, tag="cx")
                cxB = cxp.tile([P, W], F32, tag="cx")
                for g in range(NGRP):
                    scA = scp.tile([P, 2 * W], F32, tag="sc")
                    scB = scp.tile([P, 2 * W], F32, tag="sc")
                    for t in range(2):
                        y = 2 * g + t
                        ys = slice(P * y, P * (y + 1))
                        ts_ = slice(W * t, W * (t + 1))
                        nc.tensor.matmul(scA[:, ts_], k3T2[0:64, ys],
                                         qpT[i][0:64, :], start=True, stop=True,
                                         tile_position=(0, 0))
                        nc.tensor.matmul(scB[:, ts_], k3T2[64:128, ys],
                                         qpT[i][64:128, :], start=True, stop=True,
                                         tile_position=(64, 0))
                    pA = ptp.tile([P, 2 * W], BF16, tag="pt")
                    pB = ptp.tile([P, 2 * W], BF16, tag="pt")
                    nc.scalar.activation(pA[:], scA[:], EXP)
                    nc.scalar.activation(pB[:], scB[:], EXP)
                    for t in range(2):
                        y = 2 * g + t
                        ts_ = slice(W * t, W * (t + 1))
                        st = (g == 0 and t == 0)
                        sp = (g == NGRP - 1 and t == 1)
                        nc.tensor.matmul(cxA[0:65, :], v65[y][:], pA[:, ts_],
                                         start=st, stop=sp)
                        nc.tensor.matmul(cxB[0:65, :], v65[y][:], pB[:, ts_],
                                         start=st, stop=sp)
                    if g == 2 and i < NF - 2:
                        qproj(i + 2, msp)   # PE filler while ACT works on exp
                    if i >= 6 and g in (1, 2, 3, 4):
                        outpart(4 * (i - 6) + (g - 1))
                # normalize: ctxn[i][0:64] = cxA[0:64]/Z_A ; [64:128] = cxB/Z_B
                for h, cx in ((0, cxA), (1, cxB)):
                    zih = zi[2 * i + h]
                    zsh = zs[2 * i + h]
                    nc.vector.tensor_copy(zsh[:], cx[64:65, :])
                    nc.vector.reciprocal_approx_fast(zih[:], zsh[:])
                    zbc = msp.tile([P, W], F32, tag="mis")
                    nc.tensor.matmul(zbc[0:64, :], onesr[:],
                                     zih[:], start=True, stop=True,
                                     tile_position=(0, 0))
                    cxs = ptp.tile([64, W], BF16, tag="cbt")
                    with nc.allow_low_precision(reason="bf16 datapath"):
                        nc.vector.tensor_copy(cxs[:], cx[0:64, :])
                        if h == 0:
                            nc.vector.tensor_mul(ctxn[i][0:64, :], cxs[:],
                                                 zbc[0:64, :])
                        else:
                            cbt = ptp.tile([64, W], BF16, tag="cbt")
                            nc.vector.tensor_mul(cbt[:], cxs[:],
                                                 zbc[0:64, :])
                            nc.sync.dma_start(ctxn[i][64:128, :], cbt[:])

        # --- output projection tail: pairs 6,7 + the accumulated partials
        with tc.tile_pool(name="opps", bufs=4, space="PSUM") as opp, \
             tc.tile_pool(name="osb", bufs=4) as osb:
            for x in range(4):
                xs = slice(P * x, P * (x + 1))
                for o in range(2):
                    os_ = slice(W * o, W * (o + 1))
                    ps = opp.tile([P, W], F32, tag="op")
                    for i in (6, 7):
                        nc.tensor.matmul(ps[:], ctxn[i][:, xs], wo[i][:, os_],
                                         start=(i == 6), stop=(i == 7))
                    ot = osb.tile([P, W], F32, tag="os")
                    nc.vector.tensor_add(ot[:], ps[:], outacc[2 * x + o][:])
                    nc.sync.dma_start(OUT.ap()[xs, os_], ot[:])

    nc.compile()
    return nc


def _get_nc():
    if "nc" not in _CACHE:
        _CACHE["nc"] = _build()
    return _CACHE["nc"]


def kernel(q, kv, Wq, Wkv, Wo, w=None, _trace=False):
    import ml_dtypes
    from concourse import bass_utils

    BF = ml_dtypes.bfloat16

    q = np.asarray(q, np.float32).reshape(L, DM)
    kv = np.asarray(kv, np.float32).reshape(L, DM)
    Wq = np.asarray(Wq, np.float32)
    Wkv = np.asarray(Wkv, np.float32)
    Wo = np.asarray(Wo, np.float32)

    qT = np.ascontiguousarray(q.T).astype(BF)            # [DM, L]
    kvT = np.ascontiguousarray(kv.T).astype(BF)          # [DM, L]
    WQs = np.ascontiguousarray(Wq / np.sqrt(DH)).astype(BF)  # fold 1/sqrt(d_head)
    WVK = np.ascontiguousarray(
        np.concatenate([Wkv[:, DH:], Wkv[:, :DH]], axis=1)).astype(BF)  # [Wv | Wk]
    WOc = np.ascontiguousarray(Wo).astype(BF)

    in_maps = []
    for c in range(NCORES):
        kvt_c = np.zeros((DM, YW), BF)
        lo = (c - 1) * CH
        hi = (c + 2) * CH
        src_lo, src_hi = max(lo, 0), min(hi, L)
        dst_lo = src_lo - lo
        kvt_c[:, dst_lo:dst_lo + (src_hi - src_lo)] = kvT[:, src_lo:src_hi]
        in_maps.append({
            "QT": np.ascontiguousarray(qT[:, c * CH:(c + 1) * CH]),
            "KVT": kvt_c,
            "WQ": WQs,
            "WVK": WVK,
            "WO": WOc,
        })

    nc = _get_nc()
    res = bass_utils.run_bass_kernel_spmd(
        nc, in_maps, core_ids=list(range(NCORES)), trace=_trace)
    if _trace:
        _CACHE["last_result"] = res

    out = np.concatenate([r["OUT"] for r in res.results], axis=0)
    return out.reshape(B, L, DM).astype(np.float32)


# revision 22
# speedup vs baseline: 1.2081x; 1.2081x over previous
"""Local (windowed) attention with shared KV head — TRN2 Bass kernel.

Problem: b=1, L=4096, d_model=1024, n_head=16, d_head=64, w=512.
  qp = (q@Wq)/8; k,v = kv@Wkv; per 512-chunk attention over {prev,self,next}
  chunks with zero-padded edges (softmax includes exp(0)=1 terms for pads);
  out = ctx @ Wo.

Sharding: sequence-parallel over the 8 chunks, one chunk per NeuronCore.
Each core recomputes the K/V projection for its 3-chunk halo (no
collectives). Edge cores receive zero-filled halo slices, which reproduces
the reference's zero-padding exactly (scores 0 -> exp 1 in the softmax).

v2 (this file): all-bf16 datapath (f32 PSUM accumulation), fast softmax
normalization (reciprocal_approx_fast on the PSUM denominator row + DVE
partition-broadcast multiply), PE warm-up matmuls under the initial DMA
wait, q-projection interleaved into the attention loop, v-transposes via
the DMA XBAR instead of the PE.

Per-core dataflow:
  kvp^T = [Wv|Wk]^T @ kv^T            (24 MMs)   -> vT (rows 0:64), kT (64:128)
  k3T2  = kT duplicated to both partition halves (SBUF->SBUF DMA)
  v65   = DMA-transpose(vT) with a ones column appended   ([y,64+1] tiles)
  qp^T  = (Wq/8)^T @ q^T              (64 MMs)   -> 8 tiles [128,512], head pair per tile
  scores: S^T[y,x] per head, row-packed pairs (2 heads share the PE array)
  P^T   = exp(S^T) on ScalarE, PSUM->SBUF bf16, [128,1024] groups
  ctx^T+Z = [v|1]^T @ P^T fused       (M=65: rows 0:64 ctx, row 64 = softmax denom)
  norm  : zinv=recip_approx(Z row); ctxn = ctx * zinv (partition-broadcast)
  out   = ctxn^T-tiles (lhsT) @ Wo    (64 MMs)   -> [512,1024] row-major -> DMA
"""

import numpy as np

B, L, DM, NH, DH, W = 1, 4096, 1024, 16, 64, 512
NCORES = 8
CH = L // NCORES        # 512 tokens per core
YW = 3 * W              # 1536 halo positions
P = 128
NF = DM // P            # 8 feature tiles
NY = YW // P            # 12 y tiles
NPAIR = NH // 2         # 8 head pairs
NGRP = NY // 2          # 6 score groups of 2 y-tiles

_CACHE = {}


def _build():
    import concourse.mybir as mybir
    import concourse.tile as tile
    from concourse import bacc
    from concourse.masks import make_identity
    from contextlib import ExitStack

    F32 = mybir.dt.float32
    F32R = mybir.dt.float32r
    BF16 = mybir.dt.bfloat16
    EXP = mybir.ActivationFunctionType.Exp

    nc = bacc.Bacc("TRN2", target_bir_lowering=False, debug=False)
    QT = nc.dram_tensor("QT", [DM, CH], BF16, kind="ExternalInput")
    KVT = nc.dram_tensor("KVT", [DM, YW], BF16, kind="ExternalInput")
    WQ = nc.dram_tensor("WQ", [DM, DM], BF16, kind="ExternalInput")     # pre-scaled by 1/8
    WVK = nc.dram_tensor("WVK", [DM, P], BF16, kind="ExternalInput")    # [Wv | Wk]
    WO = nc.dram_tensor("WO", [DM, DM], BF16, kind="ExternalInput")
    OUT = nc.dram_tensor("OUT", [CH, DM], F32, kind="ExternalOutput")

    with tile.TileContext(nc) as tc, ExitStack() as ctx:
        perm = ctx.enter_context(tc.tile_pool(name="perm", bufs=1))

        identb = perm.tile([P, P], BF16, tag="identb")
        make_identity(nc, identb[:])
        warmsb = perm.tile([P, W], BF16, tag="warmsb")
        nc.vector.memset(warmsb[:], 1.0)
        zw = perm.tile([1, 16], F32, tag="zw")
        nc.vector.memset(zw[:], 0.0)
        onesr = perm.tile([1, 64], F32, tag="onesr")
        nc.vector.memset(onesr[:], 1.0)
        zwo = perm.tile([1, 16], F32, tag="zwo")
        # early exp-table load on ScalarE (runs during the initial DMA wait)
        nc.scalar.activation(zwo[:], zw[:], EXP)

        # --- persistent SBUF tiles
        wvk = [perm.tile([P, P], BF16, tag=f"wvk{f}", name=f"wvk{f}") for f in range(NF)]
        wq = [perm.tile([P, DM], BF16, tag=f"wq{f}", name=f"wq{f}") for f in range(NF)]
        wo = [perm.tile([P, DM], BF16, tag=f"wo{f}", name=f"wo{f}") for f in range(NF)]
        qt = [perm.tile([P, CH], BF16, tag=f"qt{f}", name=f"qt{f}") for f in range(NF)]
        k3T2 = perm.tile([P, YW], BF16, tag="k3T2")
        vTs = perm.tile([64, YW], BF16, tag="vTs")
        v65 = [perm.tile([P, 65], BF16, tag=f"v65_{t}", name=f"v65_{t}") for t in range(NY)]
        qpT = [perm.tile([P, CH], BF16, tag=f"qpT{m}", name=f"qpT{m}") for m in range(NF)]
        ctxn = [perm.tile([P, CH], BF16, tag=f"ctxn{i}", name=f"ctxn{i}") for i in range(NPAIR)]
        zi = [perm.tile([1, W], F32, tag=f"zi{h}", name=f"zi{h}") for h in range(NH)]
        zs = [perm.tile([1, W], F32, tag=f"zs{h}", name=f"zs{h}") for h in range(NH)]
        outacc = [perm.tile([P, W], F32, tag=f"oa{j}", name=f"oa{j}")
                  for j in range(8)]

        for f in range(NF):
            nc.sync.dma_start(wvk[f][:], WVK.ap()[P * f:P * (f + 1), :])

        with tc.tile_pool(name="kvt", bufs=1) as kvtp, \
             tc.tile_pool(name="warm", bufs=1, space="PSUM") as wmp, \
             tc.tile_pool(name="tpps", bufs=2, space="PSUM") as tpp, \
             tc.tile_pool(name="ph0ps", bufs=2, space="PSUM") as ph0:
            kvt = [kvtp.tile([P, YW], BF16, tag=f"kvt{f}", name=f"kvt{f}") for f in range(NF)]
            # issue ALL input DMAs up front so the sync queue never blocks a
            # load behind compute-dependent work; KVT (needed first) leads,
            # then QT and the first q-projection's WQ columns
            for n in range(3):
                for f in range(NF):
                    ns = slice(W * n, W * (n + 1))
                    nc.sync.dma_start(kvt[f][:, ns], KVT.ap()[P * f:P * (f + 1), ns])
            for f in range(NF):
                nc.sync.dma_start(qt[f][:], QT.ap()[P * f:P * (f + 1), :])
            # bulk weight loads issue from the (otherwise idle) GpSimd queue so
            # they never serialize behind critical DMAs on the sync engine
            for f in range(NF):
                nc.gpsimd.dma_start(wq[f][:], WQ.ap()[P * f:P * (f + 1), :])
            for f in range(NF):
                nc.gpsimd.dma_start(wo[f][:], WO.ap()[P * f:P * (f + 1), :])
            # PE warm-up: dense accumulating matmuls over dummy data keep the
            # HAM activity monitor busy while the KVT DMA lands (K=8/8 sooner)
            wps = wmp.tile([P, W], F32, tag="wps")
            for k in range(16):
                nc.tensor.matmul(wps[:], identb[:], warmsb[:],
                                 start=(k == 0), stop=(k == 15))
            # kv projection: [128,512] psum per n-tile; rows 0:64=vT, 64:128=kT
            for n in range(3):
                ps = ph0.tile([P, W], F32, tag="kvp")
                for f in range(NF):
                    nc.tensor.matmul(ps[:], wvk[f][:], kvt[f][:, W * n:W * (n + 1)],
                                     start=(f == 0), stop=(f == NF - 1))
                ns = slice(W * n, W * (n + 1))
                with nc.allow_low_precision(reason="bf16 datapath"):
                    nc.vector.tensor_copy(vTs[:, ns], ps[0:64, :])
                    nc.vector.tensor_copy(k3T2[64:128, ns], ps[64:128, :])
                # v65 tiles for this chunk: PE transpose of vT slices
                for t in range(4 * n, 4 * n + 4):
                    tp = tpp.tile([P, 64], BF16, tag="tp")
                    nc.tensor.transpose(tp[:], vTs[:, P * t:P * (t + 1)],
                                        identb[0:64, 0:64])
                    nc.vector.tensor_copy(v65[t][:, 0:64], tp[:])
                    nc.vector.memset(v65[t][:, 64:65], 1.0)
            # duplicate kT into the low partition half (partition remap DMA)
            nc.sync.dma_start(k3T2[0:64, :], k3T2[64:128, :])
            # second warm-up burst: bridges the PE-idle window between the kv
            # projection and the q projection (QT/WQ DMA still in flight)
            wps2 = wmp.tile([P, W], F32, tag="wps2")
            for k in range(12):
                nc.tensor.matmul(wps2[:], identb[:], warmsb[:],
                                 start=(k == 0), stop=(k == 11))

        def qproj(m, pool):
            ps = pool.tile([P, CH], F32, tag="mis")
            for f in range(NF):
                nc.tensor.matmul(ps[:], wq[f][:, P * m:P * (m + 1)], qt[f][:],
                                 start=(f == 0), stop=(f == NF - 1))
            with nc.allow_low_precision(reason="bf16 datapath"):
                nc.vector.tensor_copy(qpT[m][:], ps[:])

        # --- q projection for the first two pairs
        with tc.tile_pool(name="qpps", bufs=2, space="PSUM") as qpp:
            qproj(0, qpp)
            qproj(1, qpp)

        # --- attention per head pair; remaining q projections interleaved
        with tc.tile_pool(name="scps", bufs=2, space="PSUM") as scp, \
             tc.tile_pool(name="cxps", bufs=3, space="PSUM") as cxp, \
             tc.tile_pool(name="msps", bufs=1, space="PSUM") as msp, \
             tc.tile_pool(name="pt", bufs=4) as ptp:
            def outpart(j):
                # partial output projection over pairs 0..5 (PE filler during
                # the last two pairs' attention; ACT stays the critical path)
                x, o = divmod(j, 2)
                xs = slice(P * x, P * (x + 1))
                os_ = slice(W * o, W * (o + 1))
                ps = msp.tile([P, W], F32, tag="mis")
                for ii in range(6):
                    nc.tensor.matmul(ps[:], ctxn[ii][:, xs], wo[ii][:, os_],
                                     start=(ii == 0), stop=(ii == 5))
                nc.vector.tensor_copy(outacc[j][:], ps[:])

            for i in range(NPAIR):
                cxA = cxp.tile([P, W], F32, tag="cx")
                cxB = cxp.tile([P, W], F32, tag="cx")
                for g in range(NGRP):
                    scA = scp.tile([P, 2 * W], F32, tag="sc")
                    scB = scp.tile([P, 2 * W], F32, tag="sc")
                    for t in range(2):
                        y = 2 * g + t
                        ys = slice(P * y, P * (y + 1))
                        ts_ = slice(W * t, W * (t + 1))
                        nc.tensor.matmul(scA[:, ts_], k3T2[0:64, ys],
                                         qpT[i][0:64, :], start=True, stop=True,
                                         tile_position=(0, 0))
                        nc.tensor.matmul(scB[:, ts_], k3T2[64:128, ys],
                                         qpT[i][64:128, :], start=True, stop=True,
                                         tile_position=(64, 0))
                    pA = ptp.tile([P, 2 * W], BF16, tag="pt")
                    pB = ptp.tile([P, 2 * W], BF16, tag="pt")
                    nc.scalar.activation(pA[:], scA[:], EXP)
                    nc.scalar.activation(pB[:], scB[:], EXP)
                    for t in range(2):
                        y = 2 * g + t
                        ts_ = slice(W * t, W * (t + 1))
                        st = (g == 0 and t == 0)
                        sp = (g == NGRP - 1 and t == 1)
                        nc.tensor.matmul(cxA[0:65, :], v65[y][:], pA[:, ts_],
                                         start=st, stop=sp)
                        nc.tensor.matmul(cxB[0:65, :], v65[y][:], pB[:, ts_],
                                         start=st, stop=sp)
                    if g == 2 and i < NF - 2:
                        qproj(i + 2, msp)   # PE filler while ACT works on exp
                    if i >= 6 and g in (1, 2, 3, 4):
                        outpart(4 * (i - 6) + (g - 1))
                # normalize: ctxn[i][0:64] = cxA[0:64]/Z_A ; [64:128] = cxB/Z_B
                for h, cx in ((0, cxA), (1, cxB)):
                    zih = zi[2 * i + h]
                    zsh = zs[2 * i + h]
                    nc.vector.tensor_copy(zsh[:], cx[64:65, :])
                    nc.vector.reciprocal_approx_fast(zih[:], zsh[:])
                    zbc = msp.tile([P, W], F32, tag="mis")
                    nc.tensor.matmul(zbc[0:64, :], onesr[:],
                                     zih[:], start=True, stop=True,
                                     tile_position=(0, 0))
                    cxs = ptp.tile([64, W], BF16, tag="cbt")
                    with nc.allow_low_precision(reason="bf16 datapath"):
                        nc.vector.tensor_copy(cxs[:], cx[0:64, :])
                        if h == 0:
                            nc.vector.tensor_mul(ctxn[i][0:64, :], cxs[:],
                                                 zbc[0:64, :])
                        else:
                            cbt = ptp.tile([64, W], BF16, tag="cbt")
                            nc.vector.tensor_mul(cbt[:], cxs[:],
                                                 zbc[0:64, :])
                            nc.gpsimd.dma_start(ctxn[i][64:128, :], cbt[:])

        # --- output projection tail: pairs 6,7 + the accumulated partials
        with tc.tile_pool(name="opps", bufs=4, space="PSUM") as opp, \
             tc.tile_pool(name="osb", bufs=4) as osb:
            for x in range(4):
                xs = slice(P * x, P * (x + 1))
                for o in range(2):
                    os_ = slice(W * o, W * (o + 1))
                    ps = opp.tile([P, W], F32, tag="op")
                    for i in (6, 7):
                        nc.tensor.matmul(ps[:], ctxn[i][:, xs], wo[i][:, os_],
                                         start=(i == 6), stop=(i == 7))
                    ot = osb.tile([P, W], F32, tag="os")
                    nc.vector.tensor_add(ot[:], ps[:], outacc[2 * x + o][:])
                    nc.sync.dma_start(OUT.ap()[xs, os_], ot[:])

    nc.compile()
    return nc


def _get_nc():
    if "nc" not in _CACHE:
        _CACHE["nc"] = _build()
    return _CACHE["nc"]


def kernel(q, kv, Wq, Wkv, Wo, w=None, _trace=False):
    import ml_dtypes
    from concourse import bass_utils

    BF = ml_dtypes.bfloat16

    q = np.asarray(q, np.float32).reshape(L, DM)
    kv = np.asarray(kv, np.float32).reshape(L, DM)
    Wq = np.asarray(Wq, np.float32)
    Wkv = np.asarray(Wkv, np.float32)
    Wo = np.asarray(Wo, np.float32)

    qT = np.ascontiguousarray(q.T).astype(BF)            # [DM, L]
    kvT = np.ascontiguousarray(kv.T).astype(BF)          # [DM, L]
    WQs = np.ascontiguousarray(Wq / np.sqrt(DH)).astype(BF)  # fold 1/sqrt(d_head)
    WVK = np.ascontiguousarray(
        np.concatenate([Wkv[:, DH:], Wkv[:, :DH]], axis=1)).astype(BF)  # [Wv | Wk]
    WOc = np.ascontiguousarray(Wo).astype(BF)

    in_maps = []
    for c in range(NCORES):
        kvt_c = np.zeros((DM, YW), BF)
        lo = (c - 1) * CH
        hi = (c + 2) * CH
        src_lo, src_hi = max(lo, 0), min(hi, L)
        dst_lo = src_lo - lo
        kvt_c[:, dst_lo:dst_lo + (src_hi - src_lo)] = kvT[:, src_lo:src_hi]
        in_maps.append({
            "QT": np.ascontiguousarray(qT[:, c * CH:(c + 1) * CH]),
            "KVT": kvt_c,
            "WQ": WQs,
            "WVK": WVK,
            "WO": WOc,
        })

    nc = _get_nc()
    res = bass_utils.run_bass_kernel_spmd(
        nc, in_maps, core_ids=list(range(NCORES)), trace=_trace)
    if _trace:
        _CACHE["last_result"] = res

    out = np.concatenate([r["OUT"] for r in res.results], axis=0)
    return out.reshape(B, L, DM).astype(np.float32)


# revision 28
# speedup vs baseline: 1.2206x; 1.0104x over previous
"""Local (windowed) attention with shared KV head — TRN2 Bass kernel.

Problem: b=1, L=4096, d_model=1024, n_head=16, d_head=64, w=512.
  qp = (q@Wq)/8; k,v = kv@Wkv; per 512-chunk attention over {prev,self,next}
  chunks with zero-padded edges (softmax includes exp(0)=1 terms for pads);
  out = ctx @ Wo.

Sharding: sequence-parallel over the 8 chunks, one chunk per NeuronCore.
Each core recomputes the K/V projection for its 3-chunk halo (no
collectives). Edge cores receive zero-filled halo slices, which reproduces
the reference's zero-padding exactly (scores 0 -> exp 1 in the softmax).

v2 (this file): all-bf16 datapath (f32 PSUM accumulation), fast softmax
normalization (reciprocal_approx_fast on the PSUM denominator row + DVE
partition-broadcast multiply), PE warm-up matmuls under the initial DMA
wait, q-projection interleaved into the attention loop, v-transposes via
the DMA XBAR instead of the PE.

Per-core dataflow:
  kvp^T = [Wv|Wk]^T @ kv^T            (24 MMs)   -> vT (rows 0:64), kT (64:128)
  k3T2  = kT duplicated to both partition halves (SBUF->SBUF DMA)
  v65   = DMA-transpose(vT) with a ones column appended   ([y,64+1] tiles)
  qp^T  = (Wq/8)^T @ q^T              (64 MMs)   -> 8 tiles [128,512], head pair per tile
  scores: S^T[y,x] per head, row-packed pairs (2 heads share the PE array)
  P^T   = exp(S^T) on ScalarE, PSUM->SBUF bf16, [128,1024] groups
  ctx^T+Z = [v|1]^T @ P^T fused       (M=65: rows 0:64 ctx, row 64 = softmax denom)
  norm  : zinv=recip_approx(Z row); ctxn = ctx * zinv (partition-broadcast)
  out   = ctxn^T-tiles (lhsT) @ Wo    (64 MMs)   -> [512,1024] row-major -> DMA
"""

import numpy as np

B, L, DM, NH, DH, W = 1, 4096, 1024, 16, 64, 512
NCORES = 8
CH = L // NCORES        # 512 tokens per core
YW = 3 * W              # 1536 halo positions
P = 128
NF = DM // P            # 8 feature tiles
NY = YW // P            # 12 y tiles
NPAIR = NH // 2         # 8 head pairs
NGRP = NY // 2          # 6 score groups of 2 y-tiles

_CACHE = {}


def _build():
    import concourse.mybir as mybir
    import concourse.tile as tile
    from concourse import bacc
    from concourse.masks import make_identity
    from contextlib import ExitStack

    F32 = mybir.dt.float32
    F32R = mybir.dt.float32r
    BF16 = mybir.dt.bfloat16
    EXP = mybir.ActivationFunctionType.Exp

    nc = bacc.Bacc("TRN2", target_bir_lowering=False, debug=False)
    QT = nc.dram_tensor("QT", [DM, CH], BF16, kind="ExternalInput")
    KVT = nc.dram_tensor("KVT", [DM, YW], BF16, kind="ExternalInput")
    WQ = nc.dram_tensor("WQ", [DM, DM], BF16, kind="ExternalInput")     # pre-scaled by 1/8
    WVK = nc.dram_tensor("WVK", [DM, P], BF16, kind="ExternalInput")    # [Wv | Wk]
    WO = nc.dram_tensor("WO", [DM, DM], BF16, kind="ExternalInput")
    OUT = nc.dram_tensor("OUT", [CH, DM], F32, kind="ExternalOutput")

    with tile.TileContext(nc) as tc, ExitStack() as ctx:
        perm = ctx.enter_context(tc.tile_pool(name="perm", bufs=1))

        identb = perm.tile([P, P], BF16, tag="identb")
        make_identity(nc, identb[:])
        warmsb = perm.tile([P, W], BF16, tag="warmsb")
        nc.vector.memset(warmsb[:], 1.0)
        zw = perm.tile([1, 16], F32, tag="zw")
        nc.vector.memset(zw[:], 0.0)
        onesr = perm.tile([1, 64], F32, tag="onesr")
        nc.vector.memset(onesr[:], 1.0)
        zwo = perm.tile([1, 16], F32, tag="zwo")
        # early exp-table load on ScalarE (runs during the initial DMA wait)
        nc.scalar.activation(zwo[:], zw[:], EXP)

        # --- persistent SBUF tiles
        wvk = [perm.tile([P, P], BF16, tag=f"wvk{f}", name=f"wvk{f}") for f in range(NF)]
        wq = [perm.tile([P, DM], BF16, tag=f"wq{f}", name=f"wq{f}") for f in range(NF)]
        wo = [perm.tile([P, DM], BF16, tag=f"wo{f}", name=f"wo{f}") for f in range(NF)]
        qt = [perm.tile([P, CH], BF16, tag=f"qt{f}", name=f"qt{f}") for f in range(NF)]
        k3T2 = perm.tile([P, YW], BF16, tag="k3T2")
        vTs = perm.tile([64, YW], BF16, tag="vTs")
        v65 = [perm.tile([P, 65], BF16, tag=f"v65_{t}", name=f"v65_{t}") for t in range(NY)]
        qpT = [perm.tile([P, CH], BF16, tag=f"qpT{m}", name=f"qpT{m}") for m in range(NF)]
        ctxn = [perm.tile([P, CH], BF16, tag=f"ctxn{i}", name=f"ctxn{i}") for i in range(NPAIR)]
        zi = [perm.tile([1, W], F32, tag=f"zi{h}", name=f"zi{h}") for h in range(NH)]
        zs = [perm.tile([1, W], F32, tag=f"zs{h}", name=f"zs{h}") for h in range(NH)]
        outacc = [perm.tile([P, W], F32, tag=f"oa{j}", name=f"oa{j}")
                  for j in range(8)]

        for f in range(NF):
            nc.sync.dma_start(wvk[f][:], WVK.ap()[P * f:P * (f + 1), :])

        def qproj(m, pool):
            ps = pool.tile([P, CH], F32, tag="mis")
            for f in range(NF):
                nc.tensor.matmul(ps[:], wq[f][:, P * m:P * (m + 1)], qt[f][:],
                                 start=(f == 0), stop=(f == NF - 1))
            with nc.allow_low_precision(reason="bf16 datapath"):
                nc.vector.tensor_copy(qpT[m][:], ps[:])

        with tc.tile_pool(name="kvt", bufs=1) as kvtp, \
             tc.tile_pool(name="warm", bufs=1, space="PSUM") as wmp, \
             tc.tile_pool(name="tpps", bufs=2, space="PSUM") as tpp, \
             tc.tile_pool(name="qpps", bufs=2, space="PSUM") as qpp, \
             tc.tile_pool(name="ph0ps", bufs=2, space="PSUM") as ph0:
            kvt = [kvtp.tile([P, YW], BF16, tag=f"kvt{f}", name=f"kvt{f}") for f in range(NF)]
            # issue ALL input DMAs up front so the sync queue never blocks a
            # load behind compute-dependent work; KVT (needed first) leads,
            # then QT and the first q-projection's WQ columns
            for n in range(3):
                for f in range(NF):
                    ns = slice(W * n, W * (n + 1))
                    nc.sync.dma_start(kvt[f][:, ns], KVT.ap()[P * f:P * (f + 1), ns])
            for f in range(NF):
                nc.sync.dma_start(qt[f][:], QT.ap()[P * f:P * (f + 1), :])
            # bulk weight loads issue from the (otherwise idle) GpSimd queue so
            # they never serialize behind critical DMAs on the sync engine;
            # the first q-projection's WQ columns lead
            for f in range(NF):
                nc.gpsimd.dma_start(wq[f][:, 0:2 * P],
                                    WQ.ap()[P * f:P * (f + 1), 0:2 * P])
            for f in range(NF):
                nc.gpsimd.dma_start(wq[f][:, 2 * P:],
                                    WQ.ap()[P * f:P * (f + 1), 2 * P:])
            for f in range(NF):
                nc.gpsimd.dma_start(wo[f][:], WO.ap()[P * f:P * (f + 1), :])
            # PE warm-up: dense accumulating matmuls over dummy data keep the
            # HAM activity monitor busy while the KVT DMA lands (K=8/8 sooner)
            wps = wmp.tile([P, W], F32, tag="wps")
            for k in range(10):
                nc.tensor.matmul(wps[:], identb[:], warmsb[:],
                                 start=(k == 0), stop=(k == 9))
            # kv projection: [128,512] psum per n-tile; rows 0:64=vT, 64:128=kT
            for n in range(3):
                ps = ph0.tile([P, W], F32, tag="kvp")
                for f in range(NF):
                    nc.tensor.matmul(ps[:], wvk[f][:], kvt[f][:, W * n:W * (n + 1)],
                                     start=(f == 0), stop=(f == NF - 1))
                ns = slice(W * n, W * (n + 1))
                with nc.allow_low_precision(reason="bf16 datapath"):
                    nc.vector.tensor_copy(vTs[:, ns], ps[0:64, :])
                    nc.vector.tensor_copy(k3T2[64:128, ns], ps[64:128, :])
                # duplicate kT into the low partition half per chunk so pair-0
                # scores can start as soon as the first chunks are projected
                nc.sync.dma_start(k3T2[0:64, ns], k3T2[64:128, ns])
            # q projection for the first two pairs (before the v transposes —
            # they gate the first exp, the transposes only gate the first ctx)
            qproj(0, qpp)
            qproj(1, qpp)
            # v65 tiles: PE transpose of vT slices
            for t in range(NY):
                tp = tpp.tile([P, 64], BF16, tag="tp")
                nc.tensor.transpose(tp[:], vTs[:, P * t:P * (t + 1)],
                                    identb[0:64, 0:64])
                nc.vector.tensor_copy(v65[t][:, 0:64], tp[:])
                nc.vector.memset(v65[t][:, 64:65], 1.0)

        # --- attention per head pair; remaining q projections interleaved
        with tc.tile_pool(name="scps", bufs=2, space="PSUM") as scp, \
             tc.tile_pool(name="cxps", bufs=3, space="PSUM") as cxp, \
             tc.tile_pool(name="msps", bufs=1, space="PSUM") as msp, \
             tc.tile_pool(name="pt", bufs=4) as ptp:
            def normalize(i, cxA, cxB):
                # ctxn[i][0:64] = cxA[0:64]/Z_A ; [64:128] = cxB/Z_B
                for h, cx in ((0, cxA), (1, cxB)):
                    zih = zi[2 * i + h]
                    zsh = zs[2 * i + h]
                    nc.vector.tensor_copy(zsh[:], cx[64:65, :])
                    nc.vector.reciprocal_approx_fast(zih[:], zsh[:])
                    zbc = msp.tile([P, W], F32, tag="mis")
                    nc.tensor.matmul(zbc[0:64, :], onesr[:],
                                     zih[:], start=True, stop=True,
                                     tile_position=(0, 0))
                    cxs = ptp.tile([64, W], BF16, tag="cbt")
                    with nc.allow_low_precision(reason="bf16 datapath"):
                        nc.vector.tensor_copy(cxs[:], cx[0:64, :])
                        if h == 0:
                            nc.vector.tensor_mul(ctxn[i][0:64, :], cxs[:],
                                                 zbc[0:64, :])
                        else:
                            cbt = ptp.tile([64, W], BF16, tag="cbt")
                            nc.vector.tensor_mul(cbt[:], cxs[:],
                                                 zbc[0:64, :])
                            nc.gpsimd.dma_start(ctxn[i][64:128, :], cbt[:])

            # PE filler fragments, dispatched 1 per group so the PE queue
            # never delays the next exp's score matmuls by more than ~0.5us:
            # remaining q projections (pairs 2..7) and the partial output
            # projection over pairs 0..5 (during the last two pairs).
            filler = []   # list of closures
            qps = {}

            def qp_frag(m, lo, hi):
                def run():
                    if m not in qps:
                        qps[m] = msp.tile([P, CH], F32, tag="mis",
                                          name=f"qpf{m}")
                    ps = qps[m]
                    for f in range(lo, hi):
                        nc.tensor.matmul(ps[:], wq[f][:, P * m:P * (m + 1)],
                                         qt[f][:], start=(f == 0),
                                         stop=(f == NF - 1))
                    if hi == NF:
                        with nc.allow_low_precision(reason="bf16 datapath"):
                            nc.vector.tensor_copy(qpT[m][:], ps[:])
                        del qps[m]
                return run

            def op_frag(j, lo, hi):
                def run():
                    if j not in qps:
                        qps[j] = msp.tile([P, W], F32, tag="mis",
                                          name=f"opf{j}")
                    ps = qps[j]
                    x, o = divmod(j, 2)
                    for ii in range(lo, hi):
                        nc.tensor.matmul(ps[:], ctxn[ii][:, P * x:P * (x + 1)],
                                         wo[ii][:, W * o:W * (o + 1)],
                                         start=(ii == 0), stop=(ii == 5))
                    if hi == 6:
                        nc.vector.tensor_copy(outacc[j][:], ps[:])
                        del qps[j]
                return run

            for m in range(2, NF):
                for f0 in (0, 4):
                    filler.append(qp_frag(m, f0, f0 + 4))
            for j in range(8):
                for lo in (0, 3):
                    filler.append(op_frag(j, lo, lo + 3))
            # filler budget: 12 qp frags over pairs 0..5 (2/pair at g∈{2,4}),
            # 16 op frags over pairs 6..7 (4/pair at g∈{1,2,3,4})
            fidx = 0

            pend = None
            for i in range(NPAIR):
                cxA = cxp.tile([P, W], F32, tag="cx")
                cxB = cxp.tile([P, W], F32, tag="cx")
                for g in range(NGRP):
                    scA = scp.tile([P, 2 * W], F32, tag="sc")
                    scB = scp.tile([P, 2 * W], F32, tag="sc")
                    for t in range(2):
                        y = 2 * g + t
                        ys = slice(P * y, P * (y + 1))
                        ts_ = slice(W * t, W * (t + 1))
                        nc.tensor.matmul(scA[:, ts_], k3T2[0:64, ys],
                                         qpT[i][0:64, :], start=True, stop=True,
                                         tile_position=(0, 0))
                        nc.tensor.matmul(scB[:, ts_], k3T2[64:128, ys],
                                         qpT[i][64:128, :], start=True, stop=True,
                                         tile_position=(64, 0))
                    if g == 0 and pend is not None:
                        # deferred normalize: issued after the next pair's
                        # first score MMs so ACT never waits at pair boundary
                        normalize(*pend)
                        pend = None
                    pA = ptp.tile([P, 2 * W], BF16, tag="pt")
                    pB = ptp.tile([P, 2 * W], BF16, tag="pt")
                    nc.scalar.activation(pA[:], scA[:], EXP)
                    nc.scalar.activation(pB[:], scB[:], EXP)
                    for t in range(2):
                        y = 2 * g + t
                        ts_ = slice(W * t, W * (t + 1))
                        st = (g == 0 and t == 0)
                        sp = (g == NGRP - 1 and t == 1)
                        nc.tensor.matmul(cxA[0:65, :], v65[y][:], pA[:, ts_],
                                         start=st, stop=sp)
                        nc.tensor.matmul(cxB[0:65, :], v65[y][:], pB[:, ts_],
                                         start=st, stop=sp)
                    if i < 6:
                        if g in (2, 4) and fidx < 12:
                            filler[fidx](); fidx += 1
                    elif g in (1, 2, 3, 4):
                        if fidx < len(filler):
                            filler[fidx](); fidx += 1
                            if fidx < len(filler):
                                filler[fidx](); fidx += 1
                pend = (i, cxA, cxB)
            normalize(*pend)
            while fidx < len(filler):
                filler[fidx](); fidx += 1

        # --- output projection tail: pairs 6,7 + the accumulated partials
        with tc.tile_pool(name="opps", bufs=4, space="PSUM") as opp, \
             tc.tile_pool(name="osb", bufs=4) as osb:
            for x in range(4):
                xs = slice(P * x, P * (x + 1))
                for o in range(2):
                    os_ = slice(W * o, W * (o + 1))
                    ps = opp.tile([P, W], F32, tag="op")
                    for i in (6, 7):
                        nc.tensor.matmul(ps[:], ctxn[i][:, xs], wo[i][:, os_],
                                         start=(i == 6), stop=(i == 7))
                    ot = osb.tile([P, W], F32, tag="os")
                    nc.vector.tensor_add(ot[:], ps[:], outacc[2 * x + o][:])
                    nc.sync.dma_start(OUT.ap()[xs, os_], ot[:])

    nc.compile()
    return nc


def _get_nc():
    if "nc" not in _CACHE:
        _CACHE["nc"] = _build()
    return _CACHE["nc"]


def kernel(q, kv, Wq, Wkv, Wo, w=None, _trace=False):
    import ml_dtypes
    from concourse import bass_utils

    BF = ml_dtypes.bfloat16

    q = np.asarray(q, np.float32).reshape(L, DM)
    kv = np.asarray(kv, np.float32).reshape(L, DM)
    Wq = np.asarray(Wq, np.float32)
    Wkv = np.asarray(Wkv, np.float32)
    Wo = np.asarray(Wo, np.float32)

    qT = np.ascontiguousarray(q.T).astype(BF)            # [DM, L]
    kvT = np.ascontiguousarray(kv.T).astype(BF)          # [DM, L]
    WQs = np.ascontiguousarray(Wq / np.sqrt(DH)).astype(BF)  # fold 1/sqrt(d_head)
    WVK = np.ascontiguousarray(
        np.concatenate([Wkv[:, DH:], Wkv[:, :DH]], axis=1)).astype(BF)  # [Wv | Wk]
    WOc = np.ascontiguousarray(Wo).astype(BF)

    in_maps = []
    for c in range(NCORES):
        kvt_c = np.zeros((DM, YW), BF)
        lo = (c - 1) * CH
        hi = (c + 2) * CH
        src_lo, src_hi = max(lo, 0), min(hi, L)
        dst_lo = src_lo - lo
        kvt_c[:, dst_lo:dst_lo + (src_hi - src_lo)] = kvT[:, src_lo:src_hi]
        in_maps.append({
            "QT": np.ascontiguousarray(qT[:, c * CH:(c + 1) * CH]),
            "KVT": kvt_c,
            "WQ": WQs,
            "WVK": WVK,
            "WO": WOc,
        })

    nc = _get_nc()
    res = bass_utils.run_bass_kernel_spmd(
        nc, in_maps, core_ids=list(range(NCORES)), trace=_trace)
    if _trace:
        _CACHE["last_result"] = res

    out = np.concatenate([r["OUT"] for r in res.results], axis=0)
    return out.reshape(B, L, DM).astype(np.float32)


# revision 45
# speedup vs baseline: 1.2886x; 1.0557x over previous
"""Local (windowed) attention with shared KV head — TRN2 Bass kernel.

Problem: b=1, L=4096, d_model=1024, n_head=16, d_head=64, w=512.
  qp = (q@Wq)/8; k,v = kv@Wkv; per 512-chunk attention over {prev,self,next}
  chunks with zero-padded edges (softmax includes exp(0)=1 terms for pads);
  out = ctx @ Wo.

Sharding: sequence-parallel over the 8 chunks, one chunk per NeuronCore.
Each core recomputes the K/V projection for its 3-chunk halo (no
collectives). Edge cores receive zero-filled halo slices, which reproduces
the reference's zero-padding exactly (scores 0 -> exp 1 in the softmax).

v2 (this file): all-bf16 datapath (f32 PSUM accumulation), fast softmax
normalization (reciprocal_approx_fast on the PSUM denominator row + DVE
partition-broadcast multiply), PE warm-up matmuls under the initial DMA
wait, q-projection interleaved into the attention loop, v-transposes via
the DMA XBAR instead of the PE.

Per-core dataflow:
  kvp^T = [Wv|Wk]^T @ kv^T            (24 MMs)   -> vT (rows 0:64), kT (64:128)
  k3T2  = kT duplicated to both partition halves (SBUF->SBUF DMA)
  v65   = DMA-transpose(vT) with a ones column appended   ([y,64+1] tiles)
  qp^T  = (Wq/8)^T @ q^T              (64 MMs)   -> 8 tiles [128,512], head pair per tile
  scores: S^T[y,x] per head, row-packed pairs (2 heads share the PE array)
  P^T   = exp(S^T) on ScalarE, PSUM->SBUF bf16, [128,1024] groups
  ctx^T+Z = [v|1]^T @ P^T fused       (M=65: rows 0:64 ctx, row 64 = softmax denom)
  norm  : zinv=recip_approx(Z row); ctxn = ctx * zinv (partition-broadcast)
  out   = ctxn^T-tiles (lhsT) @ Wo    (64 MMs)   -> [512,1024] row-major -> DMA
"""

import numpy as np

B, L, DM, NH, DH, W = 1, 4096, 1024, 16, 64, 512
NCORES = 8
CH = L // NCORES        # 512 tokens per core
YW = 3 * W              # 1536 halo positions
P = 128
NF = DM // P            # 8 feature tiles
NY = YW // P            # 12 y tiles
NPAIR = NH // 2         # 8 head pairs
NGRP = NY // 2          # 6 score groups of 2 y-tiles

_CACHE = {}


def _build():
    import concourse.mybir as mybir
    import concourse.tile as tile
    from concourse import bacc
    from concourse.masks import make_identity
    from contextlib import ExitStack

    F32 = mybir.dt.float32
    F32R = mybir.dt.float32r
    BF16 = mybir.dt.bfloat16
    EXP = mybir.ActivationFunctionType.Exp

    nc = bacc.Bacc("TRN2", target_bir_lowering=False, debug=False)
    QT = nc.dram_tensor("QT", [DM, CH], BF16, kind="ExternalInput")
    KVT = nc.dram_tensor("KVT", [DM, YW], BF16, kind="ExternalInput")
    WQ = nc.dram_tensor("WQ", [DM, DM], BF16, kind="ExternalInput")     # pre-scaled by 1/8
    WVK = nc.dram_tensor("WVK", [DM, P], BF16, kind="ExternalInput")    # [Wv | Wk]
    WO = nc.dram_tensor("WO", [DM, DM], BF16, kind="ExternalInput")
    OUT = nc.dram_tensor("OUT", [CH, DM], F32, kind="ExternalOutput")

    with tile.TileContext(nc) as tc, ExitStack() as ctx:
        perm = ctx.enter_context(tc.tile_pool(name="perm", bufs=1))

        identb = perm.tile([P, P], BF16, tag="identb")
        make_identity(nc, identb[:])
        warmsb = perm.tile([P, W], BF16, tag="warmsb")
        nc.vector.memset(warmsb[:], 1.0)
        zw = perm.tile([1, 16], F32, tag="zw")
        nc.vector.memset(zw[:], 0.0)
        # [1,128] f32r selectors: selA broadcasts zinvA into output partitions
        # 0:64, selB into 64:128 (two accumulating K=1 matmuls build zbc)
        selA = perm.tile([1, P], F32R, tag="selA")
        selB = perm.tile([1, P], F32R, tag="selB")
        nc.vector.memset(selA[:].bitcast(F32), 0.0)
        nc.vector.memset(selA[0:1, 0:64].bitcast(F32), 1.0)
        nc.vector.memset(selB[:].bitcast(F32), 0.0)
        nc.vector.memset(selB[0:1, 64:128].bitcast(F32), 1.0)
        zwo = perm.tile([1, 16], F32, tag="zwo")
        # early exp-table load on ScalarE (runs during the initial DMA wait)
        nc.scalar.activation(zwo[:], zw[:], EXP)

        # --- persistent SBUF tiles
        wvk = [perm.tile([P, P], BF16, tag=f"wvk{f}", name=f"wvk{f}") for f in range(NF)]
        wq = [perm.tile([P, DM], BF16, tag=f"wq{f}", name=f"wq{f}") for f in range(NF)]
        wo = [perm.tile([P, DM], BF16, tag=f"wo{f}", name=f"wo{f}") for f in range(NF)]
        qt = [perm.tile([P, CH], BF16, tag=f"qt{f}", name=f"qt{f}") for f in range(NF)]
        k3T2 = perm.tile([P, YW], BF16, tag="k3T2")
        vTs = perm.tile([64, YW], BF16, tag="vTs")
        v65 = [perm.tile([P, 65], BF16, tag=f"v65_{t}", name=f"v65_{t}") for t in range(NY)]
        qpT = [perm.tile([P, CH], BF16, tag=f"qpT{m}", name=f"qpT{m}") for m in range(NF)]
        ctxn = [perm.tile([P, CH], BF16, tag=f"ctxn{i}", name=f"ctxn{i}") for i in range(NPAIR)]
        zsp = [perm.tile([1, W], F32, tag=f"zsp{h}", name=f"zsp{h}") for h in range(NH)]
        zif = [perm.tile([1, W], F32, tag=f"zif{h}", name=f"zif{h}") for h in range(NH)]
        zir = [perm.tile([1, W], F32R, tag=f"zir{h}", name=f"zir{h}") for h in range(NH)]
        outacc = [perm.tile([P, W], F32, tag=f"oa{j}", name=f"oa{j}")
                  for j in range(8)]

        for f in range(NF):
            nc.sync.dma_start(wvk[f][:], WVK.ap()[P * f:P * (f + 1), :])

        def qproj(m, pool):
            ps = pool.tile([P, CH], F32, tag="mis")
            for f in range(NF):
                nc.tensor.matmul(ps[:], wq[f][:, P * m:P * (m + 1)], qt[f][:],
                                 start=(f == 0), stop=(f == NF - 1))
            with nc.allow_low_precision(reason="bf16 datapath"):
                nc.vector.tensor_copy(qpT[m][:], ps[:])

        with tc.tile_pool(name="kvt", bufs=1) as kvtp, \
             tc.tile_pool(name="warm", bufs=1, space="PSUM") as wmp, \
             tc.tile_pool(name="tpps", bufs=2, space="PSUM") as tpp, \
             tc.tile_pool(name="qpps", bufs=2, space="PSUM") as qpp, \
             tc.tile_pool(name="ph0ps", bufs=2, space="PSUM") as ph0:
            kvt = [kvtp.tile([P, YW], BF16, tag=f"kvt{f}", name=f"kvt{f}") for f in range(NF)]
            # issue ALL input DMAs up front so the sync queue never blocks a
            # load behind compute-dependent work; KVT (needed first) leads,
            # then QT and the first q-projection's WQ columns
            # DMA-issue instructions cost ~650ns each and head-of-line block
            # their queue, so the count and order below set the head latency:
            # sync gets the critical path (KVT chunk 0, then the rest + QT),
            # GpSimd gets the weights
            for f in range(NF):
                nc.sync.dma_start(kvt[f][:, 0:W], KVT.ap()[P * f:P * (f + 1), 0:W])
            for f in range(NF):
                nc.sync.dma_start(kvt[f][:, W:], KVT.ap()[P * f:P * (f + 1), W:])
            for f in range(NF):
                nc.sync.dma_start(qt[f][:], QT.ap()[P * f:P * (f + 1), :])
            for f in range(NF):
                nc.sync.dma_start(wo[f][:], WO.ap()[P * f:P * (f + 1), :])
            for f in range(NF):
                nc.gpsimd.dma_start(wq[f][:, 0:2 * P],
                                    WQ.ap()[P * f:P * (f + 1), 0:2 * P])
            for f in range(NF):
                nc.gpsimd.dma_start(wq[f][:, 2 * P:],
                                    WQ.ap()[P * f:P * (f + 1), 2 * P:])
            # PE warm-up: dense accumulating matmuls over dummy data keep the
            # HAM activity monitor busy while the KVT DMA lands (K=8/8 sooner)
            wps = wmp.tile([P, W], F32, tag="wps")
            for k in range(10):
                nc.tensor.matmul(wps[:], identb[:], warmsb[:],
                                 start=(k == 0), stop=(k == 9))
            # kv projection: [128,512] psum per n-tile; rows 0:64=vT, 64:128=kT
            for n in range(3):
                ps = ph0.tile([P, W], F32, tag="kvp")
                for f in range(NF):
                    nc.tensor.matmul(ps[:], wvk[f][:], kvt[f][:, W * n:W * (n + 1)],
                                     start=(f == 0), stop=(f == NF - 1))
                ns = slice(W * n, W * (n + 1))
                with nc.allow_low_precision(reason="bf16 datapath"):
                    nc.vector.tensor_copy(vTs[:, ns], ps[0:64, :])
                    nc.vector.tensor_copy(k3T2[64:128, ns], ps[64:128, :])
                # duplicate kT into the low partition half per chunk so pair-0
                # scores can start as soon as the first chunks are projected
                # (issued on the GpSimd DGE queue; sync is busy with loads)
                nc.gpsimd.dma_start(k3T2[0:64, ns], k3T2[64:128, ns])
            # q projection for the first two pairs (before the v transposes —
            # they gate the first exp, the transposes only gate the first ctx)
            qproj(0, qpp)
            qproj(1, qpp)
            # v65 tiles: PE transpose of vT slices
            for t in range(NY):
                tp = tpp.tile([P, 64], BF16, tag="tp")
                nc.tensor.transpose(tp[:], vTs[:, P * t:P * (t + 1)],
                                    identb[0:64, 0:64])
                nc.vector.tensor_copy(v65[t][:, 0:64], tp[:])
                nc.vector.memset(v65[t][:, 64:65], 1.0)

        # --- attention per head pair; remaining q projections interleaved
        with tc.tile_pool(name="scps", bufs=2, space="PSUM") as scp, \
             tc.tile_pool(name="cxps", bufs=3, space="PSUM") as cxp, \
             tc.tile_pool(name="msps", bufs=1, space="PSUM") as msp, \
             tc.tile_pool(name="pt", bufs=4) as ptp:
            def normalize(i, cxA, cxB):
                # ctxn[i][0:64] = cxA[0:64]/Z_A ; [64:128] = cxB/Z_B
                zbc = msp.tile([P, W], F32, tag="mis")
                for h, cx, sel in ((0, cxA, selA), (1, cxB, selB)):
                    zsh, zih, zrh = zsp[2 * i + h], zif[2 * i + h], zir[2 * i + h]
                    nc.vector.tensor_copy(zsh[:], cx[64:65, :])
                    nc.vector.reciprocal_approx_fast(zih[:], zsh[:])
                    with nc.allow_low_precision(reason="f32r broadcast matmul"):
                        nc.vector.tensor_copy(zrh[:], zih[:])
                    nc.tensor.matmul(zbc[:], sel[:], zrh[:],
                                     start=(h == 0), stop=(h == 1),
                                     tile_position=(0, 0))
                cxs = ptp.tile([P, W], BF16, tag="cbt")
                with nc.allow_low_precision(reason="bf16 datapath"):
                    nc.vector.tensor_copy(cxs[0:64, :], cxA[0:64, :])
                    nc.vector.tensor_copy(cxs[64:128, :], cxB[0:64, :])
                    nc.vector.tensor_mul(ctxn[i][:], cxs[:], zbc[:])

            # PE filler fragments, dispatched 1 per group so the PE queue
            # never delays the next exp's score matmuls by more than ~0.5us:
            # remaining q projections (pairs 2..7) and the partial output
            # projection over pairs 0..5 (during the last two pairs).
            filler = []   # list of closures
            qps = {}

            def qp_frag(m, lo, hi):
                def run():
                    if m not in qps:
                        qps[m] = msp.tile([P, CH], F32, tag="mis",
                                          name=f"qpf{m}")
                    ps = qps[m]
                    for f in range(lo, hi):
                        nc.tensor.matmul(ps[:], wq[f][:, P * m:P * (m + 1)],
                                         qt[f][:], start=(f == 0),
                                         stop=(f == NF - 1))
                    if hi == NF:
                        with nc.allow_low_precision(reason="bf16 datapath"):
                            nc.vector.tensor_copy(qpT[m][:], ps[:])
                        del qps[m]
                return run

            def noop():
                pass

            def op_frag(j, lo, hi):
                def run():
                    if j not in qps:
                        qps[j] = msp.tile([P, W], F32, tag="mis",
                                          name=f"opf{j}")
                    ps = qps[j]
                    x, o = divmod(j, 2)
                    for ii in range(lo, hi):
                        nc.tensor.matmul(ps[:], ctxn[ii][:, P * x:P * (x + 1)],
                                         wo[ii][:, W * o:W * (o + 1)],
                                         start=(ii == 0), stop=(ii == 5))
                    if hi == 6:
                        nc.vector.tensor_copy(outacc[j][:], ps[:])
                        del qps[j]
                return run

            for m in range(2, NF):
                for f0 in (0, 2, 4, 6):
                    filler.append(qp_frag(m, f0, f0 + 2))
            # filler budget: 24 qp frags (2 MMs each) over pairs 0..5 at
            # g∈{1,2,3,4}; 16 op frags (3 MMs) over pairs 6..7 at g∈{1..4}
            for j in range(8):
                for lo in (0, 3):
                    filler.append(op_frag(j, lo, lo + 3))
            fidx = 0

            pend = None
            for i in range(NPAIR):
                cxA = cxp.tile([P, W], F32, tag="cx")
                cxB = cxp.tile([P, W], F32, tag="cx")
                for g in range(NGRP):
                    scA = scp.tile([P, 2 * W], F32, tag="sc")
                    scB = scp.tile([P, 2 * W], F32, tag="sc")
                    for t in range(2):
                        y = 2 * g + t
                        ys = slice(P * y, P * (y + 1))
                        ts_ = slice(W * t, W * (t + 1))
                        nc.tensor.matmul(scA[:, ts_], k3T2[0:64, ys],
                                         qpT[i][0:64, :], start=True, stop=True,
                                         tile_position=(0, 0))
                        nc.tensor.matmul(scB[:, ts_], k3T2[64:128, ys],
                                         qpT[i][64:128, :], start=True, stop=True,
                                         tile_position=(64, 0))
                    if g == 0 and pend is not None:
                        # deferred normalize: issued after the next pair's
                        # first score MMs so ACT never waits at pair boundary
                        normalize(*pend)
                        pend = None
                    pA = ptp.tile([P, 2 * W], BF16, tag="pt")
                    pB = ptp.tile([P, 2 * W], BF16, tag="pt")
                    nc.scalar.activation(pA[:], scA[:], EXP)
                    nc.scalar.activation(pB[:], scB[:], EXP)
                    for t in range(2):
                        y = 2 * g + t
                        ts_ = slice(W * t, W * (t + 1))
                        st = (g == 0 and t == 0)
                        sp = (g == NGRP - 1 and t == 1)
                        nc.tensor.matmul(cxA[0:65, :], v65[y][:], pA[:, ts_],
                                         start=st, stop=sp)
                        nc.tensor.matmul(cxB[0:65, :], v65[y][:], pB[:, ts_],
                                         start=st, stop=sp)
                    if i < 6:
                        if g in (1, 2, 3, 4) and fidx < 24:
                            filler[fidx](); fidx += 1
                    elif g in (1, 2, 3, 4):
                        if fidx < len(filler):
                            filler[fidx](); fidx += 1
                            if fidx < len(filler):
                                filler[fidx](); fidx += 1
                pend = (i, cxA, cxB)
            normalize(*pend)
            while fidx < len(filler):
                filler[fidx](); fidx += 1

        # --- output projection tail: pairs 6,7 + the accumulated partials
        with tc.tile_pool(name="opps", bufs=4, space="PSUM") as opp, \
             tc.tile_pool(name="osb", bufs=4) as osb:
            for x in range(4):
                xs = slice(P * x, P * (x + 1))
                for o in range(2):
                    os_ = slice(W * o, W * (o + 1))
                    ps = opp.tile([P, W], F32, tag="op")
                    for i in (6, 7):
                        nc.tensor.matmul(ps[:], ctxn[i][:, xs], wo[i][:, os_],
                                         start=(i == 6), stop=(i == 7))
                    ot = osb.tile([P, W], F32, tag="os")
                    nc.vector.tensor_add(ot[:], ps[:], outacc[2 * x + o][:])
                    eng = nc.sync if (2 * x + o) % 2 == 0 else nc.gpsimd
                    eng.dma_start(OUT.ap()[xs, os_], ot[:])

    nc.compile()
    return nc


def _get_nc():
    if "nc" not in _CACHE:
        _CACHE["nc"] = _build()
    return _CACHE["nc"]


def kernel(q, kv, Wq, Wkv, Wo, w=None, _trace=False):
    import ml_dtypes
    from concourse import bass_utils

    BF = ml_dtypes.bfloat16

    q = np.asarray(q, np.float32).reshape(L, DM)
    kv = np.asarray(kv, np.float32).reshape(L, DM)
    Wq = np.asarray(Wq, np.float32)
    Wkv = np.asarray(Wkv, np.float32)
    Wo = np.asarray(Wo, np.float32)

    qT = np.ascontiguousarray(q.T).astype(BF)            # [DM, L]
    kvT = np.ascontiguousarray(kv.T).astype(BF)          # [DM, L]
    WQs = np.ascontiguousarray(Wq / np.sqrt(DH)).astype(BF)  # fold 1/sqrt(d_head)
    WVK = np.ascontiguousarray(
        np.concatenate([Wkv[:, DH:], Wkv[:, :DH]], axis=1)).astype(BF)  # [Wv | Wk]
    WOc = np.ascontiguousarray(Wo).astype(BF)

    in_maps = []
    for c in range(NCORES):
        kvt_c = np.zeros((DM, YW), BF)
        lo = (c - 1) * CH
        hi = (c + 2) * CH
        src_lo, src_hi = max(lo, 0), min(hi, L)
        dst_lo = src_lo - lo
        kvt_c[:, dst_lo:dst_lo + (src_hi - src_lo)] = kvT[:, src_lo:src_hi]
        in_maps.append({
            "QT": np.ascontiguousarray(qT[:, c * CH:(c + 1) * CH]),
            "KVT": kvt_c,
            "WQ": WQs,
            "WVK": WVK,
            "WO": WOc,
        })

    nc = _get_nc()
    res = bass_utils.run_bass_kernel_spmd(
        nc, in_maps, core_ids=list(range(NCORES)), trace=_trace)
    if _trace:
        _CACHE["last_result"] = res

    out = np.concatenate([r["OUT"] for r in res.results], axis=0)
    return out.reshape(B, L, DM).astype(np.float32)


# revision 50
# speedup vs baseline: 1.3062x; 1.0137x over previous
"""Local (windowed) attention with shared KV head — TRN2 Bass kernel.

Problem: b=1, L=4096, d_model=1024, n_head=16, d_head=64, w=512.
  qp = (q@Wq)/8; k,v = kv@Wkv; per 512-chunk attention over {prev,self,next}
  chunks with zero-padded edges (softmax includes exp(0)=1 terms for pads);
  out = ctx @ Wo.

Sharding: sequence-parallel over the 8 chunks, one chunk per NeuronCore.
Each core recomputes the K/V projection for its 3-chunk halo (no
collectives). Edge cores receive zero-filled halo slices, which reproduces
the reference's zero-padding exactly (scores 0 -> exp 1 in the softmax).

v2 (this file): all-bf16 datapath (f32 PSUM accumulation), fast softmax
normalization (reciprocal_approx_fast on the PSUM denominator row + DVE
partition-broadcast multiply), PE warm-up matmuls under the initial DMA
wait, q-projection interleaved into the attention loop, v-transposes via
the DMA XBAR instead of the PE.

Per-core dataflow:
  kvp^T = [Wv|Wk]^T @ kv^T            (24 MMs)   -> vT (rows 0:64), kT (64:128)
  k3T2  = kT duplicated to both partition halves (SBUF->SBUF DMA)
  v65   = DMA-transpose(vT) with a ones column appended   ([y,64+1] tiles)
  qp^T  = (Wq/8)^T @ q^T              (64 MMs)   -> 8 tiles [128,512], head pair per tile
  scores: S^T[y,x] per head, row-packed pairs (2 heads share the PE array)
  P^T   = exp(S^T) on ScalarE, PSUM->SBUF bf16, [128,1024] groups
  ctx^T+Z = [v|1]^T @ P^T fused       (M=65: rows 0:64 ctx, row 64 = softmax denom)
  norm  : zinv=recip_approx(Z row); ctxn = ctx * zinv (partition-broadcast)
  out   = ctxn^T-tiles (lhsT) @ Wo    (64 MMs)   -> [512,1024] row-major -> DMA
"""

import numpy as np

B, L, DM, NH, DH, W = 1, 4096, 1024, 16, 64, 512
NCORES = 8
CH = L // NCORES        # 512 tokens per core
YW = 3 * W              # 1536 halo positions
P = 128
NF = DM // P            # 8 feature tiles
NY = YW // P            # 12 y tiles
NPAIR = NH // 2         # 8 head pairs
NGRP = NY // 2          # 6 score groups of 2 y-tiles

_CACHE = {}


def _build():
    import concourse.mybir as mybir
    import concourse.tile as tile
    from concourse import bacc
    from concourse.masks import make_identity
    from contextlib import ExitStack

    F32 = mybir.dt.float32
    F32R = mybir.dt.float32r
    BF16 = mybir.dt.bfloat16
    EXP = mybir.ActivationFunctionType.Exp

    nc = bacc.Bacc("TRN2", target_bir_lowering=False, debug=False)
    QT = nc.dram_tensor("QT", [DM, CH], BF16, kind="ExternalInput")
    KVT = nc.dram_tensor("KVT", [DM, YW], BF16, kind="ExternalInput")
    WQ = nc.dram_tensor("WQ", [DM, DM], BF16, kind="ExternalInput")     # pre-scaled by 1/8
    WVK = nc.dram_tensor("WVK", [DM, P], BF16, kind="ExternalInput")    # [Wv | Wk]
    WO = nc.dram_tensor("WO", [DM, DM], BF16, kind="ExternalInput")
    OUT = nc.dram_tensor("OUT", [CH, DM], F32, kind="ExternalOutput")

    with tile.TileContext(nc) as tc, ExitStack() as ctx:
        perm = ctx.enter_context(tc.tile_pool(name="perm", bufs=1))

        identb = perm.tile([P, P], BF16, tag="identb")
        make_identity(nc, identb[:])
        warmsb = perm.tile([P, W], BF16, tag="warmsb")
        nc.vector.memset(warmsb[:], 1.0)
        zw = perm.tile([1, 16], F32, tag="zw")
        nc.vector.memset(zw[:], 0.0)
        # [1,128] f32r selectors: selA broadcasts zinvA into output partitions
        # 0:64, selB into 64:128 (two accumulating K=1 matmuls build zbc)
        selA = perm.tile([1, P], F32R, tag="selA")
        selB = perm.tile([1, P], F32R, tag="selB")
        nc.vector.memset(selA[:].bitcast(F32), 0.0)
        nc.vector.memset(selA[0:1, 0:64].bitcast(F32), 1.0)
        nc.vector.memset(selB[:].bitcast(F32), 0.0)
        nc.vector.memset(selB[0:1, 64:128].bitcast(F32), 1.0)
        zwo = perm.tile([1, 16], F32, tag="zwo")
        # early exp-table load on ScalarE (runs during the initial DMA wait)
        nc.scalar.activation(zwo[:], zw[:], EXP)

        # --- persistent SBUF tiles
        wvk = [perm.tile([P, P], BF16, tag=f"wvk{f}", name=f"wvk{f}") for f in range(NF)]
        wq = [perm.tile([P, DM], BF16, tag=f"wq{f}", name=f"wq{f}") for f in range(NF)]
        wo = [perm.tile([P, DM], BF16, tag=f"wo{f}", name=f"wo{f}") for f in range(NF)]
        qt = [perm.tile([P, CH], BF16, tag=f"qt{f}", name=f"qt{f}") for f in range(NF)]
        k3T2 = perm.tile([P, YW], BF16, tag="k3T2")
        vTs = perm.tile([64, YW], BF16, tag="vTs")
        v65 = [perm.tile([P, 65], BF16, tag=f"v65_{t}", name=f"v65_{t}") for t in range(NY)]
        qpT = [perm.tile([P, CH], BF16, tag=f"qpT{m}", name=f"qpT{m}") for m in range(NF)]
        ctxn = [perm.tile([P, CH], BF16, tag=f"ctxn{i}", name=f"ctxn{i}") for i in range(NPAIR)]
        zsp = [perm.tile([1, W], F32, tag=f"zsp{h}", name=f"zsp{h}") for h in range(NH)]
        zif = [perm.tile([1, W], F32, tag=f"zif{h}", name=f"zif{h}") for h in range(NH)]
        zir = [perm.tile([1, W], F32R, tag=f"zir{h}", name=f"zir{h}") for h in range(NH)]
        outacc = [perm.tile([P, W], F32, tag=f"oa{j}", name=f"oa{j}")
                  for j in range(8)]

        for f in range(NF):
            nc.sync.dma_start(wvk[f][:], WVK.ap()[P * f:P * (f + 1), :])

        def qproj(m, pool):
            ps = pool.tile([P, CH], F32, tag="mis")
            for f in range(NF):
                nc.tensor.matmul(ps[:], wq[f][:, P * m:P * (m + 1)], qt[f][:],
                                 start=(f == 0), stop=(f == NF - 1))
            with nc.allow_low_precision(reason="bf16 datapath"):
                nc.vector.tensor_copy(qpT[m][:], ps[:])

        with tc.tile_pool(name="kvt", bufs=1) as kvtp, \
             tc.tile_pool(name="warm", bufs=1, space="PSUM") as wmp, \
             tc.tile_pool(name="tpps", bufs=2, space="PSUM") as tpp, \
             tc.tile_pool(name="qpps", bufs=2, space="PSUM") as qpp, \
             tc.tile_pool(name="ph0ps", bufs=2, space="PSUM") as ph0:
            kvt = [kvtp.tile([P, YW], BF16, tag=f"kvt{f}", name=f"kvt{f}") for f in range(NF)]
            # issue ALL input DMAs up front so the sync queue never blocks a
            # load behind compute-dependent work; KVT (needed first) leads,
            # then QT and the first q-projection's WQ columns
            # DMA-issue instructions cost ~650ns each and head-of-line block
            # their queue, so the count and order below set the head latency:
            # sync gets the critical path (KVT chunk 0, then the rest + QT),
            # GpSimd gets the weights
            # a single DMA ring moves ~150GB/s, so spread the loads over the
            # three DGE queues: sync + scalar split KVT (the critical path),
            # scalar then carries QT, GpSimd takes the weights
            for f in range(0, NF, 2):
                nc.sync.dma_start(kvt[f][:, 0:W], KVT.ap()[P * f:P * (f + 1), 0:W])
            for f in range(1, NF, 2):
                nc.scalar.dma_start(kvt[f][:, 0:W], KVT.ap()[P * f:P * (f + 1), 0:W])
            for f in range(0, NF, 2):
                nc.sync.dma_start(kvt[f][:, W:], KVT.ap()[P * f:P * (f + 1), W:])
            for f in range(1, NF, 2):
                nc.scalar.dma_start(kvt[f][:, W:], KVT.ap()[P * f:P * (f + 1), W:])
            for f in range(NF):
                nc.scalar.dma_start(qt[f][:], QT.ap()[P * f:P * (f + 1), :])
            for f in range(NF):
                nc.sync.dma_start(wo[f][:], WO.ap()[P * f:P * (f + 1), :])
            for f in range(NF):
                nc.gpsimd.dma_start(wq[f][:, 0:2 * P],
                                    WQ.ap()[P * f:P * (f + 1), 0:2 * P])
            for f in range(NF):
                nc.gpsimd.dma_start(wq[f][:, 2 * P:],
                                    WQ.ap()[P * f:P * (f + 1), 2 * P:])
            # PE warm-up: dense accumulating matmuls over dummy data keep the
            # HAM activity monitor busy while the KVT DMA lands (K=8/8 sooner)
            wps = wmp.tile([P, W], F32, tag="wps")
            for k in range(8):
                nc.tensor.matmul(wps[:], identb[:], warmsb[:],
                                 start=(k == 0), stop=(k == 7))
            # kv projection: [128,512] psum per n-tile; rows 0:64=vT, 64:128=kT
            for n in range(3):
                ps = ph0.tile([P, W], F32, tag="kvp")
                for f in range(NF):
                    nc.tensor.matmul(ps[:], wvk[f][:], kvt[f][:, W * n:W * (n + 1)],
                                     start=(f == 0), stop=(f == NF - 1))
                ns = slice(W * n, W * (n + 1))
                with nc.allow_low_precision(reason="bf16 datapath"):
                    nc.vector.tensor_copy(vTs[:, ns], ps[0:64, :])
                    nc.vector.tensor_copy(k3T2[64:128, ns], ps[64:128, :])
                # duplicate kT into the low partition half per chunk so pair-0
                # scores can start as soon as the first chunks are projected
                # (issued on the GpSimd DGE queue; sync is busy with loads)
                nc.gpsimd.dma_start(k3T2[0:64, ns], k3T2[64:128, ns])
            # v65 tiles: PE transpose of vT slices (fills the PE while the
            # QT DMA lands; only the first ctx depends on them)
            for t in range(NY):
                tp = tpp.tile([P, 64], BF16, tag="tp")
                nc.tensor.transpose(tp[:], vTs[:, P * t:P * (t + 1)],
                                    identb[0:64, 0:64])
                nc.vector.tensor_copy(v65[t][:, 0:64], tp[:])
                nc.vector.memset(v65[t][:, 64:65], 1.0)
            # q projection for the first two pairs
            qproj(0, qpp)
            qproj(1, qpp)

        # --- attention per head pair; remaining q projections interleaved
        with tc.tile_pool(name="scps", bufs=2, space="PSUM") as scp, \
             tc.tile_pool(name="cxps", bufs=3, space="PSUM") as cxp, \
             tc.tile_pool(name="msps", bufs=1, space="PSUM") as msp, \
             tc.tile_pool(name="pt", bufs=4) as ptp:
            def normalize(i, cxA, cxB):
                # ctxn[i][0:64] = cxA[0:64]/Z_A ; [64:128] = cxB/Z_B
                zbc = msp.tile([P, W], F32, tag="mis")
                for h, cx, sel in ((0, cxA, selA), (1, cxB, selB)):
                    zsh, zih, zrh = zsp[2 * i + h], zif[2 * i + h], zir[2 * i + h]
                    nc.vector.tensor_copy(zsh[:], cx[64:65, :])
                    nc.vector.reciprocal_approx_fast(zih[:], zsh[:])
                    with nc.allow_low_precision(reason="f32r broadcast matmul"):
                        nc.vector.tensor_copy(zrh[:], zih[:])
                    nc.tensor.matmul(zbc[:], sel[:], zrh[:],
                                     start=(h == 0), stop=(h == 1),
                                     tile_position=(0, 0))
                cxs = ptp.tile([P, W], BF16, tag="cbt")
                with nc.allow_low_precision(reason="bf16 datapath"):
                    nc.vector.tensor_copy(cxs[0:64, :], cxA[0:64, :])
                    nc.vector.tensor_copy(cxs[64:128, :], cxB[0:64, :])
                    nc.vector.tensor_mul(ctxn[i][:], cxs[:], zbc[:])

            # PE filler fragments, dispatched 1 per group so the PE queue
            # never delays the next exp's score matmuls by more than ~0.5us:
            # remaining q projections (pairs 2..7) and the partial output
            # projection over pairs 0..5 (during the last two pairs).
            filler = []   # list of closures
            qps = {}

            def qp_frag(m, lo, hi):
                def run():
                    if m not in qps:
                        qps[m] = msp.tile([P, CH], F32, tag="mis",
                                          name=f"qpf{m}")
                    ps = qps[m]
                    for f in range(lo, hi):
                        nc.tensor.matmul(ps[:], wq[f][:, P * m:P * (m + 1)],
                                         qt[f][:], start=(f == 0),
                                         stop=(f == NF - 1))
                    if hi == NF:
                        with nc.allow_low_precision(reason="bf16 datapath"):
                            nc.vector.tensor_copy(qpT[m][:], ps[:])
                        del qps[m]
                return run

            def noop():
                pass

            def op_frag(j, lo, hi):
                def run():
                    if j not in qps:
                        qps[j] = msp.tile([P, W], F32, tag="mis",
                                          name=f"opf{j}")
                    ps = qps[j]
                    x, o = divmod(j, 2)
                    for ii in range(lo, hi):
                        nc.tensor.matmul(ps[:], ctxn[ii][:, P * x:P * (x + 1)],
                                         wo[ii][:, W * o:W * (o + 1)],
                                         start=(ii == 0), stop=(ii == 5))
                    if hi == 6:
                        nc.vector.tensor_copy(outacc[j][:], ps[:])
                        del qps[j]
                return run

            for m in range(2, NF):
                for f0 in (0, 4):
                    filler.append(qp_frag(m, f0, f0 + 4))
            # filler budget: 12 qp frags (4 MMs each) over pairs 0..5 at
            # g∈{2,4}; 16 op frags (3 MMs) over pairs 6..7, one per group
            for j in range(8):
                for lo in (0, 3):
                    filler.append(op_frag(j, lo, lo + 3))
            fidx = 0

            pend = None
            for i in range(NPAIR):
                cxA = cxp.tile([P, W], F32, tag="cx")
                cxB = cxp.tile([P, W], F32, tag="cx")
                for g in range(NGRP):
                    scA = scp.tile([P, 2 * W], F32, tag="sc")
                    scB = scp.tile([P, 2 * W], F32, tag="sc")
                    for t in range(2):
                        y = 2 * g + t
                        ys = slice(P * y, P * (y + 1))
                        ts_ = slice(W * t, W * (t + 1))
                        nc.tensor.matmul(scA[:, ts_], k3T2[0:64, ys],
                                         qpT[i][0:64, :], start=True, stop=True,
                                         tile_position=(0, 0))
                        nc.tensor.matmul(scB[:, ts_], k3T2[64:128, ys],
                                         qpT[i][64:128, :], start=True, stop=True,
                                         tile_position=(64, 0))
                    if g == 0 and pend is not None:
                        # deferred normalize: issued after the next pair's
                        # first score MMs so ACT never waits at pair boundary
                        normalize(*pend)
                        pend = None
                    pA = ptp.tile([P, 2 * W], BF16, tag="pt")
                    pB = ptp.tile([P, 2 * W], BF16, tag="pt")
                    nc.scalar.activation(pA[:], scA[:], EXP)
                    nc.scalar.activation(pB[:], scB[:], EXP)
                    for t in range(2):
                        y = 2 * g + t
                        ts_ = slice(W * t, W * (t + 1))
                        st = (g == 0 and t == 0)
                        sp = (g == NGRP - 1 and t == 1)
                        nc.tensor.matmul(cxA[0:65, :], v65[y][:], pA[:, ts_],
                                         start=st, stop=sp)
                        nc.tensor.matmul(cxB[0:65, :], v65[y][:], pB[:, ts_],
                                         start=st, stop=sp)
                    if i < 6:
                        if g in (2, 4) and fidx < 12:
                            filler[fidx](); fidx += 1
                    else:
                        if fidx < len(filler):
                            filler[fidx](); fidx += 1
                pend = (i, cxA, cxB)
            normalize(*pend)
            while fidx < len(filler):
                filler[fidx](); fidx += 1

        # --- output projection tail: pairs 6,7 + the accumulated partials
        with tc.tile_pool(name="opps", bufs=4, space="PSUM") as opp, \
             tc.tile_pool(name="osb", bufs=4) as osb:
            for x in range(4):
                xs = slice(P * x, P * (x + 1))
                for o in range(2):
                    os_ = slice(W * o, W * (o + 1))
                    ps = opp.tile([P, W], F32, tag="op")
                    for i in (6, 7):
                        nc.tensor.matmul(ps[:], ctxn[i][:, xs], wo[i][:, os_],
                                         start=(i == 6), stop=(i == 7))
                    ot = osb.tile([P, W], F32, tag="os")
                    nc.vector.tensor_add(ot[:], ps[:], outacc[2 * x + o][:])
                    eng = nc.sync if (2 * x + o) % 2 == 0 else nc.gpsimd
                    eng.dma_start(OUT.ap()[xs, os_], ot[:])

    nc.compile()
    return nc


def _get_nc():
    if "nc" not in _CACHE:
        _CACHE["nc"] = _build()
    return _CACHE["nc"]


def kernel(q, kv, Wq, Wkv, Wo, w=None, _trace=False):
    import ml_dtypes
    from concourse import bass_utils

    BF = ml_dtypes.bfloat16

    q = np.asarray(q, np.float32).reshape(L, DM)
    kv = np.asarray(kv, np.float32).reshape(L, DM)
    Wq = np.asarray(Wq, np.float32)
    Wkv = np.asarray(Wkv, np.float32)
    Wo = np.asarray(Wo, np.float32)

    qT = np.ascontiguousarray(q.T).astype(BF)            # [DM, L]
    kvT = np.ascontiguousarray(kv.T).astype(BF)          # [DM, L]
    WQs = np.ascontiguousarray(Wq / np.sqrt(DH)).astype(BF)  # fold 1/sqrt(d_head)
    WVK = np.ascontiguousarray(
        np.concatenate([Wkv[:, DH:], Wkv[:, :DH]], axis=1)).astype(BF)  # [Wv | Wk]
    WOc = np.ascontiguousarray(Wo).astype(BF)

    in_maps = []
    for c in range(NCORES):
        kvt_c = np.zeros((DM, YW), BF)
        lo = (c - 1) * CH
        hi = (c + 2) * CH
        src_lo, src_hi = max(lo, 0), min(hi, L)
        dst_lo = src_lo - lo
        kvt_c[:, dst_lo:dst_lo + (src_hi - src_lo)] = kvT[:, src_lo:src_hi]
        in_maps.append({
            "QT": np.ascontiguousarray(qT[:, c * CH:(c + 1) * CH]),
            "KVT": kvt_c,
            "WQ": WQs,
            "WVK": WVK,
            "WO": WOc,
        })

    nc = _get_nc()
    res = bass_utils.run_bass_kernel_spmd(
        nc, in_maps, core_ids=list(range(NCORES)), trace=_trace)
    if _trace:
        _CACHE["last_result"] = res

    out = np.concatenate([r["OUT"] for r in res.results], axis=0)
    return out.reshape(B, L, DM).astype(np.float32)
